# revision 4
# baseline (speedup 1.0000x reference)
"""DiffConv (graph diffusion convolution) Trainium2 kernel.

Math (reference):
    out = sum_{k=0..2} A^k @ (H @ Wf[k]) + (A^T)^k @ (H @ Wb[k]) + bias
with H [b=8, t=24, n=1024, d=64], A [t, n, n], Wf/Wb [3, d, d].

Factorization used here (per t, batches packed into the free dim):
    U0 = H @ (Wf0 + Wb0) + bias          (computed on HOST, exact fp32 —
                                          it dominates the output; the
                                          A-chain terms are ~70x smaller)
    Uk = H @ Wf[k],  Vk = H @ Wb[k]      (on-chip, fp32r)
    out = U0 + A @ (U1 + A @ U2) + A^T @ (V1 + A^T @ V2)   (Horner)

Sharding: the t axis (24 diffusion steps) is embarrassingly parallel and
indexes both A and H -> shard t across the 8 cores (3 each). No
collectives; A is never replicated across cores (batch-sharding would
re-read the 100 MB A tensor on every core).

All matmuls run in fp32r (full PE rate, ~2^-13 relative rounding vs
fp32's 4x slowdown). A^T tiles for the forward direction are produced by
a host-side transpose (DMA transpose is 2-byte-only on TRN2).

Layouts on chip (per t):
    X^T   ht  [128, 4096]  partition=(b%2)*64+d, free=(b//2)*1024+n
    A,A^T     [128, 8, 1024] partition=row%128, free=(rowblk, col)
    U/V   uv  [128, nb(8), w(4), b(8), d(64)]  w: 0=U1 1=U2 2=V1 3=V2
    out   osb [128, i(8), (b,d)(512)]  seeded by host U0, accumulated
"""

import sys

sys.path.insert(0, "/opt/trn_rl_repo")

import numpy as np

import concourse.tile as tile
from concourse import bacc, mybir
from concourse.bass_utils import run_bass_kernel_spmd

B, T, N, D = 8, 24, 1024, 64
NCORES = 8
TPC = T // NCORES  # t-steps per core
NB = N // 128  # 128-row blocks per n dim
F32 = mybir.dt.float32
F32R = mybir.dt.float32r

_cached = {}


def _build():
    """Build + compile the per-core Bass module (SPMD: same NEFF, 8 cores)."""
    if "nc" in _cached:
        return _cached["nc"]

    nc = bacc.Bacc("TRN2", target_bir_lowering=False, debug=False)
    dHT = nc.dram_tensor("HT", [B, TPC, D, N], F32, kind="ExternalInput")
    dAT = nc.dram_tensor("AT", [TPC, N, N], F32, kind="ExternalInput")
    dA = nc.dram_tensor("Amat", [TPC, N, N], F32, kind="ExternalInput")
    dW = nc.dram_tensor("Wcat", [D, 4 * D], F32, kind="ExternalInput")
    dU0 = nc.dram_tensor("U0", [B, TPC, N, D], F32, kind="ExternalInput")
    dOUT = nc.dram_tensor("out", [B, TPC, N, D], F32, kind="ExternalOutput")

    with tile.TileContext(nc) as tc:
        with (
            tc.tile_pool(name="wc", bufs=1) as wpool,
            tc.tile_pool(name="amat", bufs=2) as apool,
            tc.tile_pool(name="ht", bufs=2) as hpool,
            tc.tile_pool(name="uv", bufs=1) as uvpool,
            tc.tile_pool(name="osb", bufs=2) as opool,
            tc.tile_pool(name="wps", bufs=4, space="PSUM") as wps,
            tc.tile_pool(name="sps", bufs=4, space="PSUM") as sps,
        ):
            # Wcat [64, 256] replicated onto both partition halves (fp32r)
            wc = wpool.tile([128, 4 * D], F32R)
            nc.gpsimd.dma_start(wc[0:64, :], dW.ap())
            nc.gpsimd.dma_start(wc[64:128, :], dW.ap())

            uv = uvpool.tile([128, NB, 4, B, D], F32R)

            for t in range(TPC):
                # ---- loads ----
                # partition = (b%2)*64 + d, free = (b//2)*N + n
                ht = hpool.tile([128, 4, N], F32R, tag="ht")
                for b2 in range(2):
                    nc.gpsimd.dma_start(
                        ht[b2 * 64 : (b2 + 1) * 64, :, :],
                        dHT.ap()[b2:B:2, t].rearrange("b1 d n -> d b1 n"),
                    )
                at = apool.tile([128, NB, N], F32R, tag="am")
                nc.gpsimd.dma_start(
                    at[:], dAT.ap()[t].rearrange("(j p) c -> p j c", p=128)
                )
                osb = opool.tile([128, NB, B * D], F32, tag="osb")
                for b in range(B):
                    nc.sync.dma_start(
                        osb[:, :, b * D : (b + 1) * D],
                        dU0.ap()[b, t].rearrange("(i p) d -> p i d", p=128),
                    )

                # ---- W-phase: U1,U2,V1,V2 = X @ [Wf1|Wf2|Wb1|Wb2] ----
                # lhsT = X^T tile [64, 128] (per b, n-block), rhs = Wcat rows
                for nb in range(NB):
                    for b in range(B):
                        half = (b % 2) * 64
                        ps = wps.tile([128, 4 * D], F32)
                        nc.tensor.matmul(
                            ps[:],
                            ht[half : half + 64, b // 2, nb * 128 : (nb + 1) * 128],
                            wc[half : half + 64, :],
                            start=True,
                            stop=True,
                        )
                        nc.vector.tensor_copy(uv[:, nb, :, b, :], ps[:])

                # ---- forward: osb += A @ (U1 + A @ U2) ----
                for i in range(NB):  # T_f = A @ U2 ; U1 += T_f (-> S_f)
                    ps = sps.tile([128, B * D], F32)
                    for j in range(NB):
                        nc.tensor.matmul(
                            ps[:],
                            at[:, j, i * 128 : (i + 1) * 128],
                            uv[:, j, 1],
                            start=(j == 0),
                            stop=(j == NB - 1),
                        )
                    nc.vector.tensor_add(uv[:, i, 0], ps[:], uv[:, i, 0])
                for i in range(NB):  # F = A @ S_f ; osb += F
                    ps = sps.tile([128, B * D], F32)
                    for j in range(NB):
                        nc.tensor.matmul(
                            ps[:],
                            at[:, j, i * 128 : (i + 1) * 128],
                            uv[:, j, 0],
                            start=(j == 0),
                            stop=(j == NB - 1),
                        )
                    nc.vector.tensor_add(osb[:, i], ps[:], osb[:, i])

                # ---- backward: osb += A^T @ (V1 + A^T @ V2) ----
                am = apool.tile([128, NB, N], F32R, tag="am")
                nc.gpsimd.dma_start(
                    am[:], dA.ap()[t].rearrange("(j p) c -> p j c", p=128)
                )
                for i in range(NB):  # T_b = A^T @ V2 ; V1 += T_b (-> S_b)
                    ps = sps.tile([128, B * D], F32)
                    for j in range(NB):
                        nc.tensor.matmul(
                            ps[:],
                            am[:, j, i * 128 : (i + 1) * 128],
                            uv[:, j, 3],
                            start=(j == 0),
                            stop=(j == NB - 1),
                        )
                    nc.vector.tensor_add(uv[:, i, 2], ps[:], uv[:, i, 2])
                for i in range(NB):  # Bk = A^T @ S_b ; osb += Bk
                    ps = sps.tile([128, B * D], F32)
                    for j in range(NB):
                        nc.tensor.matmul(
                            ps[:],
                            am[:, j, i * 128 : (i + 1) * 128],
                            uv[:, j, 2],
                            start=(j == 0),
                            stop=(j == NB - 1),
                        )
                    nc.vector.tensor_add(osb[:, i], ps[:], osb[:, i])

                # ---- store ----
                for b in range(B):
                    nc.sync.dma_start(
                        dOUT.ap()[b, t].rearrange("(i p) d -> p i d", p=128),
                        osb[:, :, b * D : (b + 1) * D],
                    )

    nc.compile()
    _cached["nc"] = nc
    return nc


def kernel(H, A, Wf, Wb, bias):
    H = np.ascontiguousarray(np.asarray(H, dtype=np.float32))
    A = np.ascontiguousarray(np.asarray(A, dtype=np.float32))
    Wf = np.asarray(Wf, dtype=np.float32)
    Wb = np.asarray(Wb, dtype=np.float32)
    bias = np.asarray(bias, dtype=np.float32)

    # Host prep: transposes, the exact-fp32 dominant term U0, packed weights.
    HT = np.ascontiguousarray(H.transpose(0, 1, 3, 2))  # [b,t,d,n]
    AT = np.ascontiguousarray(A.transpose(0, 2, 1))  # [t,n,n]
    U0 = H @ (Wf[0] + Wb[0]) + bias  # [b,t,n,d] fp32
    U0 = np.ascontiguousarray(U0.astype(np.float32))
    Wcat = np.ascontiguousarray(
        np.concatenate([Wf[1], Wf[2], Wb[1], Wb[2]], axis=1)
    )  # [64, 256]

    nc = _build()
    in_maps = []
    for c in range(NCORES):
        ts = slice(c * TPC, (c + 1) * TPC)
        in_maps.append(
            {
                "HT": np.ascontiguousarray(HT[:, ts]),
                "AT": np.ascontiguousarray(AT[ts]),
                "Amat": np.ascontiguousarray(A[ts]),
                "Wcat": Wcat,
                "U0": np.ascontiguousarray(U0[:, ts]),
            }
        )
    res = run_bass_kernel_spmd(nc, in_maps, core_ids=list(range(NCORES)))

    out = np.empty((B, T, N, D), dtype=np.float32)
    for c in range(NCORES):
        out[:, c * TPC : (c + 1) * TPC] = res.results[c]["out"]
    return out
